# revision 24
# baseline (speedup 1.0000x reference)
# DeepSeek MoE gate kernel for Trainium2 (Bass/Tile), 8 NeuronCores SPMD.
#
# reference semantics (fp32):
#   x = hidden_states.reshape(T, H)            T = 4*4096 = 16384, H = 2048
#   logits = x @ weight.T                       [T, E], E = 64
#   scores = softmax(logits, -1)
#   topk_weight, topk_idx = top_k(scores, 6)
#   aux_loss = seq-aux load balancing loss
#   topk_weight = topk_weight / (sum_k topk_weight + 1e-20)
#
# Strategy:
#   - Token-parallel across 8 cores (2048 tokens each, each core entirely
#     inside one batch of 4096 tokens).
#   - Host pre-transposes x to [H, T] so the x tile can be the matmul
#     stationary operand directly: psum[tokens, experts] = xT_tile.T @ wT,
#     token-major with no on-chip transposes.
#   - Top-6 per token via the DVE Max8 / MaxIndex instructions on the raw
#     logits (monotonic in softmax scores). topk_weight = exp(top logits)
#     normalized among themselves (full softmax denominator cancels; the
#     reference's +1e-20 is a no-op at fp32 for these magnitudes).
#   - Aux loss: the heavy T-dim reduction sum_t scores[t, e] runs on device
#     as a PE matmul with the per-token reciprocal softmax denominators as
#     the stationary vector.  The tiny [B, E] combine + ce counts (bincount
#     of the returned indices) happen on host as part of unsharding.
#   - Matmul schemes: "f32" (plain fp32, 4 cyc/row) and "f16x3" (x and w
#     split into fp16 value + 2^-11-scaled fp16 residual; three 1 cyc/row
#     matmuls reconstruct fp32-grade logits:
#       logits = xh.wh + 2^-11 * (xh.wl' + xl'.wh), residual scale 2^11
#     keeps every operand in fp16 normal range).

import numpy as np

import concourse.bass as bass
import concourse.tile as tile
from concourse import bacc, mybir
from concourse.bass_utils import run_bass_kernel_spmd

B = 4
SEQ = 4096
H = 2048
E = 64
K = 6
ALPHA = 0.001
T = B * SEQ                      # 16384
NCORES = 8
TC = T // NCORES                 # 2048 tokens per core
P = 128                          # partitions
NH = H // P                      # 16 contraction chunks
CHUNK = 512                      # tokens per processing chunk
NG = CHUNK // P                  # 4 token groups of 128 per chunk
NCHUNK = TC // CHUNK             # 4 chunks per core
RESID_SCALE = 2.0 ** 11

import os
SCHEME = os.environ.get("MOE_GATE_SCHEME", "f16x3")   # "f32" | "f16x3"

_BUILT = {}


def _build(scheme):
    f32 = mybir.dt.float32
    nc = bacc.Bacc("TRN2", target_bir_lowering=False, debug=False)

    if scheme == "f32":
        xt = [nc.dram_tensor("xt", [H, TC], f32, kind="ExternalInput").ap()]
        wt = [nc.dram_tensor("wt", [H, E], f32, kind="ExternalInput").ap()]
    else:
        f16 = mybir.dt.float16
        xt = [
            nc.dram_tensor("xth", [H, TC], f16, kind="ExternalInput").ap(),
            nc.dram_tensor("xtl", [H, TC], f16, kind="ExternalInput").ap(),
        ]
        # wh and wl packed in one tensor so a single DMA (single queue
        # semaphore) covers the whole gate weight: the PE matmul ISA slot
        # only fits one sync wait.
        wt = [nc.dram_tensor("wtp", [H, 2, E], f16, kind="ExternalInput").ap()]
    out_idx = nc.dram_tensor("out_idx", [TC, K], mybir.dt.uint32, kind="ExternalOutput").ap()
    out_w = nc.dram_tensor("out_w", [TC, K], f32, kind="ExternalOutput").ap()
    out_aux = nc.dram_tensor("out_aux", [1, NG * E], f32, kind="ExternalOutput").ap()

    with tile.TileContext(nc) as tc:
        _emit(tc, scheme, xt, wt, out_idx, out_w, out_aux)
    nc.compile()
    return nc


def _emit(tc, scheme, xt, wt, out_idx, out_w, out_aux):
    nc = tc.nc
    f32 = mybir.dt.float32
    X = mybir.AxisListType.X
    n_x = len(xt)  # 1 for f32, 2 for f16x3

    from contextlib import ExitStack
    ctx = ExitStack()
    with ctx:
        singles = ctx.enter_context(tc.tile_pool(name="singles", bufs=1))
        xpool = ctx.enter_context(tc.tile_pool(name="xp", bufs=2 * NH * n_x))
        lpool = ctx.enter_context(tc.tile_pool(name="lp", bufs=NCHUNK))
        epool = ctx.enter_context(tc.tile_pool(name="ep", bufs=NCHUNK))
        spool = ctx.enter_context(tc.tile_pool(name="sp", bufs=NCHUNK))
        opool = ctx.enter_context(tc.tile_pool(name="op", bufs=NCHUNK))
        pps = ctx.enter_context(tc.tile_pool(name="pps", bufs=2, space="PSUM"))
        paux = ctx.enter_context(tc.tile_pool(name="paux", bufs=1, space="PSUM"))

        # gate weight, resident.
        # f32: one [P, NH, E] tensor.  f16x3: [P, NH, 2, E] with wh and wl
        # side by side so one xh stationary load serves both via a single
        # N=2E matmul.
        if scheme == "f32":
            wt_sb = singles.tile([P, NH, E], wt[0].dtype, tag="wt0")
            nc.sync.dma_start(out=wt_sb, in_=wt[0].rearrange("(hc p) e -> p hc e", p=P))
        else:
            wt_sb = singles.tile([P, NH, 2, E], wt[0].dtype, tag="wt0")
            nc.sync.dma_start(
                out=wt_sb, in_=wt[0].rearrange("(hc p) two e -> p hc two e", p=P))

        # all-ones stationary vector for the aux column-sum matmul (DVE-
        # written so that matmul depends on a single engine)
        ones_sb = singles.tile([P, 1], f32, tag="ones")
        nc.vector.memset(ones_sb, 1.0)

        aux_ps = paux.tile([1, NG * E], f32)

        # whole-core output accumulators; DMA'd once at the end (a single
        # gpsimd pre-consume absorbs the DVE data dependency so each output
        # DMA carries at most its queue wait)
        i8_all = singles.tile([P, NCHUNK, NG, 8], mybir.dt.uint32, tag="i8a")
        w6_all = singles.tile([P, NCHUNK, NG, K], f32, tag="w6a")

        for c in range(NCHUNK):
            # ---- load x chunk: per h-chunk [P, CHUNK] slices of xT ----
            x_sb = []  # [n_x][NH]
            for i, x in enumerate(xt):
                tiles = []
                for h in range(NH):
                    t = xpool.tile([P, CHUNK], x.dtype, tag=f"x{i}")
                    nc.sync.dma_start(
                        out=t,
                        in_=x[h * P:(h + 1) * P, c * CHUNK:(c + 1) * CHUNK],
                    )
                    tiles.append(t)
                x_sb.append(tiles)

            # ---- logits into PSUM, token-major ----
            # One start/stop accumulation chain per token group: start=True
            # pending-zeroes the whole 2 KiB PSUM bank, so chains within a
            # bank must be strictly sequential (group-outer loop).
            if scheme == "f32":
                l_ps = pps.tile([P, NG, E], f32)
                for g in range(NG):
                    for h in range(NH):
                        nc.tensor.matmul(
                            l_ps[:, g, :],
                            lhsT=x_sb[0][h][:, g * P:(g + 1) * P],
                            rhs=wt_sb[:, h, :],
                            start=(h == 0),
                            stop=(h == NH - 1),
                        )
            else:
                # [:, g, 0, :] main (xh.wh), [:, g, 1, :] residual
                # (xh.wl' + xl'.wh, scale 2^11).  Both from one chain per
                # group: the xh matmul covers [wh|wl] in one N=2E pass.
                l_ps = pps.tile([P, NG, 2, E], f32)
                for g in range(NG):
                    for h in range(NH):
                        hi = x_sb[0][h][:, g * P:(g + 1) * P]
                        lo = x_sb[1][h][:, g * P:(g + 1) * P]
                        nc.tensor.matmul(
                            l_ps[:, g, :, :], lhsT=hi, rhs=wt_sb[:, h, :, :],
                            start=(h == 0), stop=False,
                        )
                        nc.tensor.matmul(
                            l_ps[:, g, 1, :], lhsT=lo, rhs=wt_sb[:, h, 0, :],
                            start=False, stop=(h == NH - 1),
                        )

            # ---- logits to SBUF ----
            # DVE-only PSUM reads: a single-engine reader set keeps the
            # next-round matmul's wait list within the 1-slot MM ISA budget.
            l_sb = lpool.tile([P, NG, E], f32, tag="l")
            if scheme == "f32":
                nc.vector.tensor_copy(out=l_sb, in_=l_ps)
            else:
                resid = lpool.tile([P, NG, E], f32, tag="resid")
                nc.vector.tensor_scalar_mul(
                    out=resid, in0=l_ps[:, :, 1, :], scalar1=1.0 / RESID_SCALE)
                nc.vector.tensor_add(out=l_sb, in0=l_ps[:, :, 0, :], in1=resid)

            # ---- softmax pieces: E = exp(l), D = rowsum, r = 1/D ----
            e_sb = epool.tile([P, NG, E], f32, tag="e")
            d_sb = spool.tile([P, NG], f32, tag="d")
            for g in range(NG):
                nc.scalar.activation(
                    out=e_sb[:, g, :], in_=l_sb[:, g, :],
                    func=mybir.ActivationFunctionType.Exp,
                    accum_out=d_sb[:, g:g + 1],
                )
            r_sb = spool.tile([P, NG], f32, tag="r")
            nc.vector.reciprocal(out=r_sb, in_=d_sb)

            # ---- top-8 values + indices per token group ----
            m8 = spool.tile([P, NG, 8], f32, tag="m8")
            for g in range(NG):
                nc.vector.max(out=m8[:, g, :], in_=l_sb[:, g, :])
            for g in range(NG):
                nc.vector.max_index(
                    out=i8_all[:, c, g, :], in_max=m8[:, g, :],
                    in_values=l_sb[:, g, :],
                )

            # ---- topk weights: v = exp(top logits), w = v / sum(v[:6]) ----
            v8 = spool.tile([P, NG, 8], f32, tag="v8")
            nc.scalar.activation(
                out=v8, in_=m8, func=mybir.ActivationFunctionType.Exp,
            )
            s6 = spool.tile([P, NG], f32, tag="s6")
            nc.vector.reduce_sum(out=s6, in_=v8[:, :, 0:K], axis=X)
            r6 = spool.tile([P, NG], f32, tag="r6")
            nc.vector.reciprocal(out=r6, in_=s6)
            for g in range(NG):
                nc.vector.tensor_scalar_mul(
                    out=w6_all[:, c, g, :], in0=v8[:, g, 0:K],
                    scalar1=r6[:, g:g + 1],
                )

            # ---- aux partial: sum_t softmax(l)[t, e] via PE column-sum ----
            # e2 = exp(l) / D (true softmax rows, DVE-written), then a
            # ones-stationary matmul sums over the 128 tokens of each group.
            e2_sb = epool.tile([P, NG, E], f32, tag="e2")
            for g in range(NG):
                nc.vector.tensor_scalar_mul(
                    out=e2_sb[:, g, :], in0=e_sb[:, g, :], scalar1=r_sb[:, g:g + 1],
                )
            nc.tensor.matmul(
                aux_ps,
                lhsT=ones_sb,
                rhs=e2_sb.rearrange("p g e -> p (g e)"),
                start=(c == 0),
                stop=(c == NCHUNK - 1),
            )

        aux_sb = singles.tile([1, NG * E], f32)
        nc.vector.tensor_copy(out=aux_sb, in_=aux_ps)

        # multi-wait DMAs are legal here: Bacc.generate_event_semaphores
        # splits waits to satisfy the 1-wait-per-instruction ISA limit.
        o_idx = out_idx.rearrange("(c g p) k -> p c g k", p=P, g=NG)
        o_w = out_w.rearrange("(c g p) k -> p c g k", p=P, g=NG)
        nc.sync.dma_start(out=o_idx, in_=i8_all[:, :, :, 0:K])
        nc.sync.dma_start(out=o_w, in_=w6_all)
        nc.sync.dma_start(out=out_aux, in_=aux_sb)


def _get_nc():
    if SCHEME not in _BUILT:
        _BUILT[SCHEME] = _build(SCHEME)
    return _BUILT[SCHEME]


def _prep_inputs(hidden_states, weight):
    x = np.asarray(hidden_states, dtype=np.float32).reshape(T, H)
    w = np.asarray(weight, dtype=np.float32)
    xT = np.ascontiguousarray(x.T)          # [H, T]
    wT = np.ascontiguousarray(w.T)          # [H, E]
    in_maps = []
    if SCHEME == "f32":
        for c in range(NCORES):
            in_maps.append({
                "xt": np.ascontiguousarray(xT[:, c * TC:(c + 1) * TC]),
                "wt": wT,
            })
    else:
        xh = xT.astype(np.float16)
        xl = ((xT - xh.astype(np.float32)) * RESID_SCALE).astype(np.float16)
        wh = wT.astype(np.float16)
        wl = ((wT - wh.astype(np.float32)) * RESID_SCALE).astype(np.float16)
        wtp = np.ascontiguousarray(np.stack([wh, wl], axis=1))   # [H, 2, E]
        for c in range(NCORES):
            in_maps.append({
                "xth": np.ascontiguousarray(xh[:, c * TC:(c + 1) * TC]),
                "xtl": np.ascontiguousarray(xl[:, c * TC:(c + 1) * TC]),
                "wtp": wtp,
            })
    return in_maps


def _assemble(results):
    topk_idx = np.concatenate(
        [r["out_idx"] for r in results], axis=0).view(np.int32)
    topk_weight = np.concatenate([r["out_w"] for r in results], axis=0)

    # per-core sum_t scores[t, e]: [1, NG*E] column sums, one block per group
    ssm_core = np.zeros((NCORES, E), dtype=np.float64)
    for cidx, r in enumerate(results):
        a = r["out_aux"]
        for g in range(NG):
            ssm_core[cidx] += a[0, g * E:(g + 1) * E]
    cores_per_b = NCORES // B
    ssm = ssm_core.reshape(B, cores_per_b, E).sum(axis=1) / SEQ   # [B, E]

    idx_b = topk_idx.reshape(B, SEQ * K)
    ce = np.zeros((B, E), dtype=np.float64)
    for b in range(B):
        ce[b] = np.bincount(idx_b[b], minlength=E)
    ce = ce / (SEQ * K / E)

    aux_loss = np.float32((ce * ssm).sum(axis=1).mean() * ALPHA)
    return topk_idx, topk_weight, aux_loss


def _run(hidden_states, weight, **kwargs):
    nc = _get_nc()
    in_maps = _prep_inputs(hidden_states, weight)
    return run_bass_kernel_spmd(nc, in_maps, core_ids=list(range(NCORES)), **kwargs)


def kernel(hidden_states, weight):
    res = _run(hidden_states, weight)
    return _assemble(res.results)


def kernel_traced(hidden_states, weight):
    """Returns (outputs, BassKernelResults) with NTFF profiling enabled."""
    res = _run(hidden_states, weight, trace=True)
    return _assemble(res.results), res


# revision 28
# speedup vs baseline: 1.3519x; 1.3519x over previous
# DeepSeek MoE gate kernel for Trainium2 (Bass/Tile), 8 NeuronCores SPMD.
#
# reference semantics (fp32):
#   x = hidden_states.reshape(T, H)            T = 4*4096 = 16384, H = 2048
#   logits = x @ weight.T                       [T, E], E = 64
#   scores = softmax(logits, -1)
#   topk_weight, topk_idx = top_k(scores, 6)
#   aux_loss = seq-aux load balancing loss
#   topk_weight = topk_weight / (sum_k topk_weight + 1e-20)
#
# Strategy:
#   - Token-parallel across 8 cores (2048 tokens each; every core's range
#     lies inside a single batch of 4096 tokens).
#   - fp32-grade logits from fp16 operands ("f16x3"): x and w split into
#     fp16 value + 2^11-scaled fp16 residual; three bf16-rate matmuls give
#       logits = xh.wh + 2^-11 * (xh.wl' + xl'.wh)
#     (error ~2^-21 per term, far below the fp32 PSUM accumulation noise).
#   - Weights-stationary matmul: the [wh|wl] pair is the 128-column
#     stationary operand, tokens stream as the 512-wide moving operand, so
#     each weight load amortizes over 512 token columns and the PE stays in
#     its back-to-back streaming regime (the x-stationary variant measured
#     253 ns/MM from per-matmul weight reloads + HAM oscillation).
#   - The [64, 512] expert-major logits are combined (main + 2^-11*resid)
#     on DVE and transposed to token-major via 4 PE transposes.
#   - Top-6 per token via the DVE Max8 / MaxIndex instructions on the raw
#     logits (monotonic in softmax scores). topk_weight = exp(top logits)
#     normalized among themselves (full softmax denominator cancels; the
#     reference's +1e-20 is a no-op at fp32 for these magnitudes).
#   - Aux loss: sum_t softmax(l)[t, e] runs on device as a ones-stationary
#     PE column-sum of the DVE-scaled softmax rows. The tiny [B, E]
#     combine + ce counts (bincount of the returned indices) happen on the
#     host as part of unsharding.
#   - Inputs are host-packed chunk-major ([NCHUNK, H, 2, CHUNK] fp16) so
#     each 512-token chunk arrives as one 4.2 MB DMA with 4 KB contiguous
#     runs (DMA efficiency needs >=1 MB transfers; the first cut used
#     64 x 256 KB DMAs and was issue-bound on the Sync engine).

import numpy as np

import concourse.bass as bass
import concourse.tile as tile
from concourse import bacc, mybir
from concourse.bass_utils import run_bass_kernel_spmd

B = 4
SEQ = 4096
H = 2048
E = 64
K = 6
ALPHA = 0.001
T = B * SEQ                      # 16384
NCORES = 8
TC = T // NCORES                 # 2048 tokens per core
P = 128                          # partitions
NH = H // P                      # 16 contraction chunks
CHUNK = 512                      # tokens per processing chunk
NG = CHUNK // P                  # 4 token groups of 128 per chunk
NCHUNK = TC // CHUNK             # 4 chunks per core
RESID_SCALE = 2.0 ** 11

_BUILT = {}


def _build():
    f32 = mybir.dt.float32
    f16 = mybir.dt.float16
    nc = bacc.Bacc("TRN2", target_bir_lowering=False, debug=False)

    xpk = nc.dram_tensor("xpk", [NCHUNK, H, 2, CHUNK], f16, kind="ExternalInput").ap()
    wtp = nc.dram_tensor("wtp", [H, 4, E], f16, kind="ExternalInput").ap()
    out_idx = nc.dram_tensor("out_idx", [TC, K], mybir.dt.uint32, kind="ExternalOutput").ap()
    out_w = nc.dram_tensor("out_w", [TC, K], f32, kind="ExternalOutput").ap()
    out_aux = nc.dram_tensor("out_aux", [1, NG * E], f32, kind="ExternalOutput").ap()

    with tile.TileContext(nc) as tc:
        _emit(tc, xpk, wtp, out_idx, out_w, out_aux)
    nc.compile()
    return nc


def _emit(tc, xpk, wtp, out_idx, out_w, out_aux):
    nc = tc.nc
    f32 = mybir.dt.float32
    f16 = mybir.dt.float16
    X = mybir.AxisListType.X

    from contextlib import ExitStack
    ctx = ExitStack()
    with ctx:
        singles = ctx.enter_context(tc.tile_pool(name="singles", bufs=1))
        xpool = ctx.enter_context(tc.tile_pool(name="xp", bufs=2))
        cpool = ctx.enter_context(tc.tile_pool(name="cp", bufs=2))
        lpool = ctx.enter_context(tc.tile_pool(name="lp", bufs=NCHUNK))
        epool = ctx.enter_context(tc.tile_pool(name="ep", bufs=NCHUNK))
        spool = ctx.enter_context(tc.tile_pool(name="sp", bufs=NCHUNK))
        praw_pool = ctx.enter_context(tc.tile_pool(name="praw", bufs=2, space="PSUM"))
        ptr_pool = ctx.enter_context(tc.tile_pool(name="ptr", bufs=2, space="PSUM"))
        paux = ctx.enter_context(tc.tile_pool(name="paux", bufs=1, space="PSUM"))

        # gate weight, resident: [P, NH, 4, E] = [wh, wl, 0, wh] per h so
        # both stationaries are full 128-column loads:
        #   slice [0:2] = [wh|wl] (xh pass), slice [2:4] = [0|wh] (xl pass)
        wt_sb = singles.tile([P, NH, 4, E], f16, tag="wt")
        nc.sync.dma_start(
            out=wt_sb, in_=wtp.rearrange("(hc p) four e -> p hc four e", p=P))

        # all-ones stationary vector for the aux column-sum matmul
        ones_sb = singles.tile([P, 1], f32, tag="ones")
        nc.vector.memset(ones_sb, 1.0)

        # identity (moving operand of PE transposes) — standard gpsimd recipe
        ident = singles.tile([64, 64], f32, tag="ident")
        nc.gpsimd.memset(ident, 0.0)
        nc.gpsimd.affine_select(
            out=ident, in_=ident,
            compare_op=mybir.AluOpType.not_equal,
            fill=1.0, base=0, pattern=[[-1, 64]], channel_multiplier=1,
        )

        aux_ps = paux.tile([1, NG * E], f32)

        # whole-core output accumulators; DMA'd once at the end
        i8_all = singles.tile([P, NCHUNK, NG, 8], mybir.dt.uint32, tag="i8a")
        w6_all = singles.tile([P, NCHUNK, NG, K], f32, tag="w6a")

        for c in range(NCHUNK):
            # ---- one 4.2 MB DMA: this chunk's xh/xl for all h ----
            xc = xpool.tile([P, NH, 2, CHUNK], f16, tag="x")
            nc.sync.dma_start(
                out=xc,
                in_=xpk[c].rearrange("(hc p) two t -> p hc two t", p=P),
            )

            # ---- expert-major logits: praw[0:64]=main, [64:128]=resid ----
            # single accumulation chain per chunk; weights stationary,
            # tokens moving (N=512)
            # every matmul covers all 128 output rows (the xl pass uses the
            # zero-padded [0|wh] stationary), so one clean start/stop chain.
            praw = praw_pool.tile([P, CHUNK], f32, tag="praw")
            for h in range(NH):
                nc.tensor.matmul(
                    praw, lhsT=wt_sb[:, h, 0:2, :], rhs=xc[:, h, 0, :],
                    start=(h == 0), stop=False,
                )
                nc.tensor.matmul(
                    praw, lhsT=wt_sb[:, h, 2:4, :], rhs=xc[:, h, 1, :],
                    start=False, stop=(h == NH - 1),
                )

            # ---- combine main + 2^-11 * resid (still expert-major) ----
            t1 = cpool.tile([64, CHUNK], f32, tag="t1")
            nc.vector.tensor_scalar_mul(
                out=t1, in0=praw[64:P, :], scalar1=1.0 / RESID_SCALE)
            comb = cpool.tile([64, CHUNK], f32, tag="comb")
            nc.vector.tensor_add(out=comb, in0=praw[0:64, :], in1=t1)

            # ---- transpose to token-major [P, NG, E] ----
            ltr = ptr_pool.tile([P, NG, E], f32, tag="ltr")
            for g in range(NG):
                nc.tensor.transpose(
                    out=ltr[:, g, :], in_=comb[:, g * P:(g + 1) * P],
                    identity=ident,
                )

            # ---- softmax pieces: e = exp(l), d = rowsum, r = 1/d ----
            e_sb = epool.tile([P, NG, E], f32, tag="e")
            d_sb = spool.tile([P, NG], f32, tag="d")
            for g in range(NG):
                nc.scalar.activation(
                    out=e_sb[:, g, :], in_=ltr[:, g, :],
                    func=mybir.ActivationFunctionType.Exp,
                    accum_out=d_sb[:, g:g + 1],
                )
            r_sb = spool.tile([P, NG], f32, tag="r")
            nc.vector.reciprocal(out=r_sb, in_=d_sb)

            # ---- logits to SBUF for the top-k unit ----
            l_sb = lpool.tile([P, NG, E], f32, tag="l")
            nc.vector.tensor_copy(out=l_sb, in_=ltr)

            # ---- top-8 values + indices per token group ----
            m8 = spool.tile([P, NG, 8], f32, tag="m8")
            for g in range(NG):
                nc.vector.max(out=m8[:, g, :], in_=l_sb[:, g, :])
            for g in range(NG):
                nc.vector.max_index(
                    out=i8_all[:, c, g, :], in_max=m8[:, g, :],
                    in_values=l_sb[:, g, :],
                )

            # ---- topk weights: v = exp(top logits), w = v / sum(v[:6]) ----
            v8 = spool.tile([P, NG, 8], f32, tag="v8")
            nc.scalar.activation(
                out=v8, in_=m8, func=mybir.ActivationFunctionType.Exp,
            )
            s6 = spool.tile([P, NG], f32, tag="s6")
            nc.vector.reduce_sum(out=s6, in_=v8[:, :, 0:K], axis=X)
            r6 = spool.tile([P, NG], f32, tag="r6")
            nc.vector.reciprocal(out=r6, in_=s6)
            for g in range(NG):
                nc.vector.tensor_scalar_mul(
                    out=w6_all[:, c, g, :], in0=v8[:, g, 0:K],
                    scalar1=r6[:, g:g + 1],
                )

            # ---- aux partial: sum_t softmax(l)[t, e] via PE column-sum ----
            e2_sb = epool.tile([P, NG, E], f32, tag="e2")
            for g in range(NG):
                nc.vector.tensor_scalar_mul(
                    out=e2_sb[:, g, :], in0=e_sb[:, g, :], scalar1=r_sb[:, g:g + 1],
                )
            nc.tensor.matmul(
                aux_ps,
                lhsT=ones_sb,
                rhs=e2_sb.rearrange("p g e -> p (g e)"),
                start=(c == 0),
                stop=(c == NCHUNK - 1),
            )

        aux_sb = singles.tile([1, NG * E], f32)
        nc.vector.tensor_copy(out=aux_sb, in_=aux_ps)

        # multi-wait DMAs are legal: Bacc.generate_event_semaphores splits
        # waits to satisfy the 1-wait-per-instruction ISA limit.
        o_idx = out_idx.rearrange("(c g p) k -> p c g k", p=P, g=NG)
        o_w = out_w.rearrange("(c g p) k -> p c g k", p=P, g=NG)
        nc.sync.dma_start(out=o_idx, in_=i8_all[:, :, :, 0:K])
        nc.sync.dma_start(out=o_w, in_=w6_all)
        nc.sync.dma_start(out=out_aux, in_=aux_sb)


def _get_nc():
    if "nc" not in _BUILT:
        _BUILT["nc"] = _build()
    return _BUILT["nc"]


def _prep_inputs(hidden_states, weight):
    x = np.asarray(hidden_states, dtype=np.float32).reshape(T, H)
    w = np.asarray(weight, dtype=np.float32)
    xT = np.ascontiguousarray(x.T)          # [H, T]
    wT = np.ascontiguousarray(w.T)          # [H, E]

    xh = xT.astype(np.float16)
    xl = ((xT - xh.astype(np.float32)) * RESID_SCALE).astype(np.float16)
    wh = wT.astype(np.float16)
    wl = ((wT - wh.astype(np.float32)) * RESID_SCALE).astype(np.float16)
    zero = np.zeros_like(wh)
    wtp = np.ascontiguousarray(np.stack([wh, wl, zero, wh], axis=1))  # [H, 4, E]

    in_maps = []
    for c in range(NCORES):
        ch = xh[:, c * TC:(c + 1) * TC].reshape(H, NCHUNK, CHUNK)
        cl = xl[:, c * TC:(c + 1) * TC].reshape(H, NCHUNK, CHUNK)
        # [NCHUNK, H, 2, CHUNK]
        xpk = np.ascontiguousarray(
            np.stack([ch, cl], axis=2).transpose(1, 0, 2, 3))
        in_maps.append({"xpk": xpk, "wtp": wtp})
    return in_maps


def _assemble(results):
    topk_idx = np.concatenate(
        [r["out_idx"] for r in results], axis=0).view(np.int32)
    topk_weight = np.concatenate([r["out_w"] for r in results], axis=0)

    # per-core sum_t scores[t, e]: [1, NG*E] column sums, one block per group
    ssm_core = np.zeros((NCORES, E), dtype=np.float64)
    for cidx, r in enumerate(results):
        a = r["out_aux"]
        for g in range(NG):
            ssm_core[cidx] += a[0, g * E:(g + 1) * E]
    cores_per_b = NCORES // B
    ssm = ssm_core.reshape(B, cores_per_b, E).sum(axis=1) / SEQ   # [B, E]

    idx_b = topk_idx.reshape(B, SEQ * K)
    ce = np.zeros((B, E), dtype=np.float64)
    for b in range(B):
        ce[b] = np.bincount(idx_b[b], minlength=E)
    ce = ce / (SEQ * K / E)

    aux_loss = np.float32((ce * ssm).sum(axis=1).mean() * ALPHA)
    return topk_idx, topk_weight, aux_loss


def _run(hidden_states, weight, **kwargs):
    nc = _get_nc()
    in_maps = _prep_inputs(hidden_states, weight)
    return run_bass_kernel_spmd(nc, in_maps, core_ids=list(range(NCORES)), **kwargs)


def kernel(hidden_states, weight):
    res = _run(hidden_states, weight)
    return _assemble(res.results)


def kernel_traced(hidden_states, weight):
    """Returns (outputs, BassKernelResults) with NTFF profiling enabled."""
    res = _run(hidden_states, weight, trace=True)
    return _assemble(res.results), res
